# revision 7
# baseline (speedup 1.0000x reference)
"""Trainium2 Bass kernel for the ComplexSSMState problem.

Full inputs in, full output out. Internally: 16 independent (g,b) slabs
sharded over 8 NeuronCores (2 slabs/core, single g per core). Per slab the
S=4096 scan runs in 512-wide chunks chained through tensor_tensor_scan
carries, phased by ACT table set (exp -> trig -> sqrt) to avoid table
thrashing.
"""

import sys
import numpy as np

sys.path.insert(0, "/opt/trn_rl_repo")

B, S, G, D, N = 4, 4096, 4, 256, 128
NCORES = 8
SLABS = 2          # slabs per core
SC = 512           # chunk length along S
f32 = np.float32

INV2PI = float(f32(1.0 / (2.0 * np.pi)))
TWOPI = float(f32(2.0 * np.pi))
MAGIC = float(f32(12582912.0))  # 1.5 * 2**23 round-to-nearest trick


def _prep_shared(inputs):
    """Host-side parameter rearrangement shared across cores."""
    Bwr = np.asarray(inputs["B_w_r"], dtype=f32)   # [N, D]
    Bwi = np.asarray(inputs["B_w_i"], dtype=f32)
    Cwr = np.asarray(inputs["C_w_r"], dtype=f32)   # [D, N]
    Cwi = np.asarray(inputs["C_w_i"], dtype=f32)
    dtw = np.asarray(inputs["dt_w"], dtype=f32)    # [2, 2D]
    dtb = np.asarray(inputs["dt_b"], dtype=f32)    # [2]

    def ktiles(a):  # [2K, M] -> [K, 2M] side-by-side k-tiles
        return np.concatenate([a[:128, :], a[128:, :]], axis=1).astype(f32)

    BwrT = Bwr.T.copy()            # [D, N] = [256,128]
    BwiT = Bwi.T.copy()
    p = {}
    p["bwrT"] = ktiles(BwrT)                       # [128, 256]
    p["bwiT"] = ktiles(BwiT)
    p["bwiTn"] = ktiles((-BwiT).astype(f32))
    dtwT = np.zeros((512, 33), dtype=f32)          # mag col 0, phase col 32
    dtwT[:, 0] = dtw.T[:, 0]
    dtwT[:, 32] = dtw.T[:, 1]
    p["dtwT"] = np.concatenate([dtwT[k * 128:(k + 1) * 128, :] for k in range(4)],
                               axis=1).astype(f32)  # [128, 132]
    p["cwcr"] = np.concatenate([Cwr.T, Cwi.T], axis=1).astype(f32)          # [128,512]
    p["cwci"] = np.concatenate([(-Cwi.T).astype(f32), Cwr.T], axis=1).astype(f32)
    dtbp = np.zeros((33, 1), dtype=f32)
    dtbp[0, 0] = dtb[0]
    dtbp[32, 0] = dtb[1]
    p["dtb"] = dtbp
    p["ident"] = np.eye(128, dtype=f32)
    p["ones_row"] = np.ones((1, 128), dtype=f32)
    return p


def _per_core(inputs, core):
    g = core // 2
    b0 = 2 * (core % 2)
    la = np.asarray(inputs["log_A_mag"], dtype=f32)[g]   # [N]
    nla = (-np.log1p(np.exp(la))).astype(f32)            # -softplus
    aph = np.asarray(inputs["A_phase"], dtype=f32)[g]
    xr = np.ascontiguousarray(np.asarray(inputs["x_real"], dtype=f32)[b0:b0 + 2, :, g, :])
    xi = np.ascontiguousarray(np.asarray(inputs["x_imag"], dtype=f32)[b0:b0 + 2, :, g, :])
    return {"xr": xr, "xi": xi,
            "nla": nla.reshape(1, N), "aph": aph.reshape(1, N)}


def _emit(nc, tile_mod, mybir, s_len, n_slabs):
    from contextlib import ExitStack
    Af = mybir.ActivationFunctionType
    Op = mybir.AluOpType
    dt32 = mybir.dt.float32
    nchunk = s_len // SC

    # ---- DRAM I/O ----
    xr_d = nc.dram_tensor("xr", [n_slabs, s_len, D], dt32, kind="ExternalInput").ap()
    xi_d = nc.dram_tensor("xi", [n_slabs, s_len, D], dt32, kind="ExternalInput").ap()
    nla_d = nc.dram_tensor("nla", [1, N], dt32, kind="ExternalInput").ap()
    aph_d = nc.dram_tensor("aph", [1, N], dt32, kind="ExternalInput").ap()
    bwrT_d = nc.dram_tensor("bwrT", [128, 256], dt32, kind="ExternalInput").ap()
    bwiT_d = nc.dram_tensor("bwiT", [128, 256], dt32, kind="ExternalInput").ap()
    bwiTn_d = nc.dram_tensor("bwiTn", [128, 256], dt32, kind="ExternalInput").ap()
    dtwT_d = nc.dram_tensor("dtwT", [128, 132], dt32, kind="ExternalInput").ap()
    cwcr_d = nc.dram_tensor("cwcr", [128, 512], dt32, kind="ExternalInput").ap()
    cwci_d = nc.dram_tensor("cwci", [128, 512], dt32, kind="ExternalInput").ap()
    dtb_d = nc.dram_tensor("dtb", [33, 1], dt32, kind="ExternalInput").ap()
    ident_d = nc.dram_tensor("ident", [128, 128], dt32, kind="ExternalInput").ap()
    ones_d = nc.dram_tensor("ones_row", [1, N], dt32, kind="ExternalInput").ap()
    y_d = nc.dram_tensor("y", [n_slabs, s_len, 2 * D], dt32, kind="ExternalOutput").ap()

    with tile_mod.TileContext(nc) as tc, ExitStack() as ctx:
        const = ctx.enter_context(tc.tile_pool(name="const", bufs=1))
        slab_p = ctx.enter_context(tc.tile_pool(name="slab", bufs=1))
        xin_p = ctx.enter_context(tc.tile_pool(name="xin", bufs=2))
        xt_p = ctx.enter_context(tc.tile_pool(name="xt", bufs=2))
        tmp_p = ctx.enter_context(tc.tile_pool(name="tmp", bufs=1))
        carry_p = ctx.enter_context(tc.tile_pool(name="carry", bufs=2))
        ps_xt = ctx.enter_context(tc.tile_pool(name="psxt", bufs=2, space="PSUM"))
        ps_outer = ctx.enter_context(tc.tile_pool(name="psout", bufs=2, space="PSUM"))
        ps_bx = ctx.enter_context(tc.tile_pool(name="psbx", bufs=2, space="PSUM"))
        ps_y = ctx.enter_context(tc.tile_pool(name="psy", bufs=2, space="PSUM"))

        def cload(name, src, shape):
            t = const.tile(shape, dt32, tag=name)
            nc.sync.dma_start(t[:], src[:])
            return t

        ident = cload("ident", ident_d, [128, 128])
        bwrT = cload("bwrT", bwrT_d, [128, 256])
        bwiT = cload("bwiT", bwiT_d, [128, 256])
        bwiTn = cload("bwiTn", bwiTn_d, [128, 256])
        dtwT = cload("dtwT", dtwT_d, [128, 132])
        cwcr = cload("cwcr", cwcr_d, [128, 512])
        cwci = cload("cwci", cwci_d, [128, 512])
        nla = cload("nla", nla_d, [1, N])
        aph = cload("aph", aph_d, [1, N])
        ones = cload("ones", ones_d, [1, N])
        dtb = cload("dtb", dtb_d, [33, 1])
        zeros = const.tile([128, SC], dt32, tag="zeros")
        nc.vector.memset(zeros[:], 0.0)
        bias8 = const.tile([128, 1], dt32, tag="bias8")
        nc.vector.memset(bias8[:], 1e-8)

        for sl in range(n_slabs):
            # full-slab SBUF buffers (tags reused across slabs)
            ea_b = slab_p.tile([128, s_len], dt32, tag="ea")
            cli_b = slab_p.tile([128, s_len], dt32, tag="cli")
            bxr_b = slab_p.tile([128, s_len], dt32, tag="bxr")  # later aliased to hr
            bxi_b = slab_p.tile([128, s_len], dt32, tag="bxi")  # later aliased to hi
            dtm_b = slab_p.tile([1, s_len], dt32, tag="dtm")
            dtp_b = slab_p.tile([1, s_len], dt32, tag="dtp")

            # ---------- Phase A: exp table ----------
            prev_clr = None
            for c in range(nchunk):
                rg = slice(c * SC, (c + 1) * SC)
                xin_r = xin_p.tile([128, 1024], dt32, tag="xinr")
                xin_i = xin_p.tile([128, 1024], dt32, tag="xini")
                nc.sync.dma_start(
                    xin_r[:].rearrange("p (sub d) -> p sub d", sub=4),
                    xr_d[sl, rg, :].rearrange("(sub p) d -> p sub d", p=128))
                nc.sync.dma_start(
                    xin_i[:].rearrange("p (sub d) -> p sub d", sub=4),
                    xi_d[sl, rg, :].rearrange("(sub p) d -> p sub d", p=128))
                xtr = xt_p.tile([128, 1024], dt32, tag="xtr")
                xti = xt_p.tile([128, 1024], dt32, tag="xti")
                for xin, xt in ((xin_r, xtr), (xin_i, xti)):
                    for dblk in range(2):
                        ps = ps_xt.tile([128, 512], dt32, tag="xtps")
                        for sub in range(4):
                            nc.tensor.transpose(
                                ps[:, sub * 128:(sub + 1) * 128],
                                xin[:, sub * 256 + dblk * 128: sub * 256 + dblk * 128 + 128],
                                ident[:])
                        nc.scalar.copy(xt[:, dblk * 512:(dblk + 1) * 512], ps[:])
                # dt projection: mag -> partition 0, phase -> partition 32
                dtps = ps_outer.tile([128, 512], dt32, tag="outer")
                rhs_k = [xtr[:, :512], xtr[:, 512:], xti[:, :512], xti[:, 512:]]
                for k in range(4):
                    nc.tensor.matmul(dtps[0:33, :], dtwT[:, 33 * k:33 * k + 33], rhs_k[k],
                                     start=(k == 0), stop=(k == 3))
                dte_m = tmp_p.tile([1, SC], dt32, tag="dtem")
                nc.scalar.activation(dte_m[:], dtps[0:1, :], Af.Exp, bias=dtb[0:1, 0:1])
                dte_p = tmp_p.tile([1, SC], dt32, tag="dtep")
                nc.scalar.activation(dte_p[:], dtps[32:33, :], Af.Exp, bias=dtb[32:33, 0:1])
                nc.vector.tensor_scalar(dtm_b[:, rg], dte_m[:], 2.0, 1e-4, Op.min, Op.max)
                nc.vector.tensor_scalar(dtp_b[:, rg], dte_p[:], 2.0, 1e-4, Op.min, Op.max)
                # outer products -> log_A chunks
                lar = ps_outer.tile([128, 512], dt32, tag="outer")
                nc.tensor.matmul(lar[:], nla[:], dtm_b[:, rg], start=True, stop=True)
                lai = ps_outer.tile([128, 512], dt32, tag="outer")
                nc.tensor.matmul(lai[:], aph[:], dtp_b[:, rg], start=True, stop=True)
                # scans (sequential carry)
                clr = carry_p.tile([128, SC], dt32, tag="clr")
                init_r = 0.0 if c == 0 else prev_clr[:, SC - 1:SC]
                nc.vector.tensor_tensor_scan(clr[:], zeros[:], lar[:], init_r, Op.add, Op.add)
                init_i = 0.0 if c == 0 else cli_b[:, c * SC - 1:c * SC]
                nc.vector.tensor_tensor_scan(cli_b[:, rg], zeros[:], lai[:], init_i,
                                             Op.add, Op.add)
                prev_clr = clr
                nc.scalar.activation(ea_b[:, rg], clr[:], Af.Exp)
                # B projection
                bxr_ps = ps_bx.tile([128, 512], dt32, tag="bx")
                nc.tensor.matmul(bxr_ps[:], bwrT[:, :128], xtr[:, :512], start=True, stop=False)
                nc.tensor.matmul(bxr_ps[:], bwrT[:, 128:], xtr[:, 512:], start=False, stop=False)
                nc.tensor.matmul(bxr_ps[:], bwiTn[:, :128], xti[:, :512], start=False, stop=False)
                nc.tensor.matmul(bxr_ps[:], bwiTn[:, 128:], xti[:, 512:], start=False, stop=True)
                nc.vector.tensor_copy(bxr_b[:, rg], bxr_ps[:])
                bxi_ps = ps_bx.tile([128, 512], dt32, tag="bx")
                nc.tensor.matmul(bxi_ps[:], bwiT[:, :128], xtr[:, :512], start=True, stop=False)
                nc.tensor.matmul(bxi_ps[:], bwiT[:, 128:], xtr[:, 512:], start=False, stop=False)
                nc.tensor.matmul(bxi_ps[:], bwrT[:, :128], xti[:, :512], start=False, stop=False)
                nc.tensor.matmul(bxi_ps[:], bwrT[:, 128:], xti[:, 512:], start=False, stop=True)
                nc.vector.tensor_copy(bxi_b[:, rg], bxi_ps[:])

            # ---------- Phase BC: trig table ----------
            prev_Qr = None
            prev_Qi = None
            for c in range(nchunk):
                rg = slice(c * SC, (c + 1) * SC)
                u = tmp_p.tile([128, SC], dt32, tag="u")
                nc.vector.tensor_scalar_mul(u[:], cli_b[:, rg], INV2PI)
                rnd = tmp_p.tile([128, SC], dt32, tag="rnd")
                nc.vector.tensor_scalar(rnd[:], u[:], MAGIC, MAGIC, Op.add, Op.subtract)
                nc.vector.tensor_tensor(u[:], u[:], rnd[:], Op.subtract)
                sC_ = tmp_p.tile([128, SC], dt32, tag="sC")
                nc.scalar.activation(sC_[:], u[:], Af.Sin, scale=TWOPI)
                v = tmp_p.tile([128, SC], dt32, tag="v")
                nc.vector.tensor_scalar(v[:], cli_b[:, rg], INV2PI, 0.25, Op.mult, Op.add)
                rnd2 = tmp_p.tile([128, SC], dt32, tag="rnd")
                nc.vector.tensor_scalar(rnd2[:], v[:], MAGIC, MAGIC, Op.add, Op.subtract)
                nc.vector.tensor_tensor(v[:], v[:], rnd2[:], Op.subtract)
                cC_ = tmp_p.tile([128, SC], dt32, tag="cC")
                nc.scalar.activation(cC_[:], v[:], Af.Sin, scale=TWOPI)
                cAr = tmp_p.tile([128, SC], dt32, tag="cAr")
                nc.vector.tensor_tensor(cAr[:], ea_b[:, rg], cC_[:], Op.mult)
                cAi = tmp_p.tile([128, SC], dt32, tag="cAi")
                nc.vector.tensor_tensor(cAi[:], ea_b[:, rg], sC_[:], Op.mult)
                dr = tmp_p.tile([128, SC], dt32, tag="dr")
                nc.vector.tensor_scalar_add(dr[:], cAr[:], 1e-12)
                sq1 = tmp_p.tile([128, SC], dt32, tag="sq1")
                nc.scalar.activation(sq1[:], dr[:], Af.Square)
                sq2 = tmp_p.tile([128, SC], dt32, tag="sq2")
                nc.scalar.activation(sq2[:], cAi[:], Af.Square)
                nc.vector.tensor_tensor(sq1[:], sq1[:], sq2[:], Op.add)
                gg = tmp_p.tile([128, SC], dt32, tag="gg")
                nc.vector.reciprocal(gg[:], sq1[:])
                dtmf = ps_outer.tile([128, 512], dt32, tag="outer")
                nc.tensor.matmul(dtmf[:], ones[:], dtm_b[:, rg], start=True, stop=True)
                nc.vector.tensor_tensor(gg[:], gg[:], dtmf[:], Op.mult)  # f = dtm*g
                t3 = tmp_p.tile([128, SC], dt32, tag="t3")
                nc.gpsimd.tensor_tensor(t3[:], bxr_b[:, rg], dr[:], Op.mult)
                t4 = tmp_p.tile([128, SC], dt32, tag="t4")
                nc.gpsimd.tensor_tensor(t4[:], bxi_b[:, rg], cAi[:], Op.mult)
                nc.gpsimd.tensor_tensor(t3[:], t3[:], t4[:], Op.add)
                nc.gpsimd.tensor_tensor(t3[:], t3[:], gg[:], Op.mult)  # qr
                t5 = tmp_p.tile([128, SC], dt32, tag="t5")
                nc.gpsimd.tensor_tensor(t5[:], bxi_b[:, rg], dr[:], Op.mult)
                t6 = tmp_p.tile([128, SC], dt32, tag="t6")
                nc.gpsimd.tensor_tensor(t6[:], bxr_b[:, rg], cAi[:], Op.mult)
                nc.gpsimd.tensor_tensor(t5[:], t5[:], t6[:], Op.subtract)
                nc.gpsimd.tensor_tensor(t5[:], t5[:], gg[:], Op.mult)  # qi
                Qr = carry_p.tile([128, SC], dt32, tag="Qr")
                init_qr = 0.0 if c == 0 else prev_Qr[:, SC - 1:SC]
                nc.vector.tensor_tensor_scan(Qr[:], zeros[:], t3[:], init_qr, Op.add, Op.add)
                Qi = carry_p.tile([128, SC], dt32, tag="Qi")
                init_qi = 0.0 if c == 0 else prev_Qi[:, SC - 1:SC]
                nc.vector.tensor_tensor_scan(Qi[:], zeros[:], t5[:], init_qi, Op.add, Op.add)
                prev_Qr, prev_Qi = Qr, Qi
                t7 = tmp_p.tile([128, SC], dt32, tag="t6")
                nc.gpsimd.tensor_tensor(t7[:], cAr[:], Qr[:], Op.mult)
                t8 = tmp_p.tile([128, SC], dt32, tag="t8")
                nc.gpsimd.tensor_tensor(t8[:], cAi[:], Qi[:], Op.mult)
                nc.gpsimd.tensor_tensor(bxr_b[:, rg], t7[:], t8[:], Op.subtract)  # hr
                t9 = tmp_p.tile([128, SC], dt32, tag="t6")
                nc.gpsimd.tensor_tensor(t9[:], cAr[:], Qi[:], Op.mult)
                t10 = tmp_p.tile([128, SC], dt32, tag="t8")
                nc.gpsimd.tensor_tensor(t10[:], cAi[:], Qr[:], Op.mult)
                nc.gpsimd.tensor_tensor(bxi_b[:, rg], t9[:], t10[:], Op.add)  # hi

            # ---------- Phase D: sqrt table ----------
            for c in range(nchunk):
                rg = slice(c * SC, (c + 1) * SC)
                m1 = tmp_p.tile([128, SC], dt32, tag="m1")
                nc.scalar.activation(m1[:], bxr_b[:, rg], Af.Square)
                m2 = tmp_p.tile([128, SC], dt32, tag="m2")
                nc.scalar.activation(m2[:], bxi_b[:, rg], Af.Square)
                nc.vector.tensor_tensor(m1[:], m1[:], m2[:], Op.add)
                hn = tmp_p.tile([128, SC], dt32, tag="hn")
                nc.scalar.activation(hn[:], m1[:], Af.Sqrt, bias=bias8[:, 0:1])
                rc = tmp_p.tile([128, SC], dt32, tag="rc")
                nc.vector.reciprocal(rc[:], hn[:])
                nc.vector.tensor_scalar(rc[:], rc[:], 100.0, 1.0, Op.mult, Op.min)
                hrp = tmp_p.tile([128, SC], dt32, tag="hrp")
                nc.gpsimd.tensor_tensor(hrp[:], bxr_b[:, rg], rc[:], Op.mult)
                hip = tmp_p.tile([128, SC], dt32, tag="hip")
                nc.gpsimd.tensor_tensor(hip[:], bxi_b[:, rg], rc[:], Op.mult)
                for sb in range(4):
                    yps = ps_y.tile([128, 512], dt32, tag="y")
                    nc.tensor.matmul(yps[:], hrp[:, sb * 128:(sb + 1) * 128], cwcr[:],
                                     start=True, stop=False)
                    nc.tensor.matmul(yps[:], hip[:, sb * 128:(sb + 1) * 128], cwci[:],
                                     start=False, stop=True)
                    ybc = tmp_p.tile([128, 512], dt32, tag="ybc")
                    nc.vector.tensor_copy(ybc[:], yps[:])
                    nc.sync.dma_start(
                        y_d[sl, c * SC + sb * 128: c * SC + (sb + 1) * 128, :], ybc[:])
    return nc


_CACHE = {}


def _build(s_len=S, n_slabs=SLABS):
    key = (s_len, n_slabs)
    if key in _CACHE:
        return _CACHE[key]
    import concourse.bacc as bacc
    import concourse.tile as tile_mod
    import concourse.mybir as mybir
    nc = bacc.Bacc("TRN2", target_bir_lowering=False, debug=False,
                   enable_asserts=False, num_devices=NCORES)
    _emit(nc, tile_mod, mybir, s_len, n_slabs)
    nc.compile()
    _CACHE[key] = nc
    return nc


def kernel(**inputs):
    from concourse.bass_utils import run_bass_kernel_spmd
    nc = _build()
    shared = _prep_shared(inputs)
    in_maps = []
    for core in range(NCORES):
        m = dict(shared)
        m.update(_per_core(inputs, core))
        in_maps.append(m)
    res = run_bass_kernel_spmd(nc, in_maps, list(range(NCORES)))
    y = np.zeros((B, S, G, 2 * D), dtype=f32)
    for core in range(NCORES):
        g = core // 2
        b0 = 2 * (core % 2)
        y[b0:b0 + 2, :, g, :] = res.results[core]["y"]
    return y
